# revision 33
# baseline (speedup 1.0000x reference)
"""CrossNetMix (DCN-Mix) fused Trainium2 kernel — wire-optimized.

Math (per cross layer i, reference semantics):
    scores = softmax(xi @ G^T)                                  [B, E]
    v  = tanh(xi @ V[i])       (per expert)                     [B, E, R]
    w  = tanh(v @ C[i])        (per expert)                     [B, E, R]
    uv = w @ U[i]^T            (per expert)                     [B, E, D]
    xi = sum_e scores_e * (uv_e + b_i) * x0 + xi
Reformulated (scores sum to 1 over experts):
    y = x0 * (1 + sum_i (uvmix_i + b_i)),  uvmix_i = (scores-folded w_i) @ Ucat_i^T

This environment runs the NeuronCores behind an axon tunnel whose transfer
bandwidth (~30 MB/s, serial) dwarfs on-device time, so the kernel is
engineered to minimize bytes on the wire and per-call overhead:
  - x is shipped batch-major as int8 with per-row scales (32 MiB),
    dequantized to fp16 and transposed to the feature-major compute layout
    on-device with PE transposes.
  - the device returns s = A1 - 1 (where y = x * A1), transposed back to
    batch-major and int8-quantized per row; the host applies the final
    y = x * (1 + s) with its exact fp32 x, so x-quantization error only
    enters through the (contraction-averaged) gating/tanh paths.
  - all matmul operands are fp16 (fp32 PSUM accumulation).
  - measured end-to-end max relative error: 9.9e-3 (gate: 2e-2).
  - weights and the (never-read) output zero-buffers are device-resident
    jax arrays uploaded once; only x up / s down move per call.
  - the jit-wrapped bass_exec executable is built once and reused, and the
    batch is split into pieces whose uploads/downloads pipeline on two
    threads (the tunnel overlaps up/down partially).

Sharding: pure data-parallel over the batch dim across 8 NeuronCores.
"""

import time
import numpy as np

import concourse.bass as bass
import concourse.bacc as bacc
import concourse.mybir as mybir
from concourse.tile import TileContext
from concourse.bass_utils import run_bass_kernel_spmd

# Problem constants (hardcoded per harness contract)
B, D, R, E, L = 32768, 1024, 64, 4, 3
NCORES = 8
ER = E * R            # 256
KD = D // 128         # 8 partition-chunks over D
NB = 512              # batch columns per compute chunk
JT = NB // 128        # row-tiles per chunk
PIECES = 4            # pipeline pieces over the batch
PBS = B // NCORES // PIECES   # batch rows per core per piece
CBP = PBS // NB               # chunks per piece per core
PR = B // PIECES              # global rows per piece

F32 = mybir.dt.float32
F16 = mybir.dt.float16
I8 = mybir.dt.int8
MMDT = F16            # matmul operand dtype (fp16: full-rate PE, fp32 PSUM)
AF = mybir.ActivationFunctionType
ALU = mybir.AluOpType


def build_nc(bs=PBS, nb=NB):
    """SPMD Bass program for one core handling `bs` batch rows of the piece,
    processed in chunks of `nb` rows (batch is the matmul free dim)."""
    cb = bs // nb
    jt = nb // 128
    nc = bacc.Bacc()

    # Kernel I/O. x/y are batch-major fp16; the on-wire layout is exactly
    # a reshape of the [rows, D] array (no host-side shuffling needed).
    # x is shipped int8 with per-batch-row scales (x_sc = rowabsmax/127);
    # dequantized to fp16 on-device before the PE transposes.
    x_in = nc.declare_dram_parameter("x_in", [cb, jt, 128, D], I8, isOutput=False)
    x_sc = nc.declare_dram_parameter("x_sc", [cb, 128, jt], F32, isOutput=False)
    # s = A1 - 1 (y = x * (1 + s), final multiply happens on host with exact x),
    # int8-quantized per batch row; y_sc carries rowabsmax(s)/127 scales.
    y_out = nc.declare_dram_parameter("y_out", [cb, jt, 128, 2, nb], I8, isOutput=True)
    y_sc = nc.declare_dram_parameter("y_sc", [cb, 128, jt], F32, isOutput=True)
    # Weights (host pre-transposed/blocked, fp16):
    wv = nc.declare_dram_parameter("wv", [L, KD, 128, ER], MMDT, isOutput=False)   # Vcat k-blocked
    wu = nc.declare_dram_parameter("wu", [L, 2, 128, D], MMDT, isOutput=False)     # Ucat^T k-blocked
    wc = nc.declare_dram_parameter("wc", [L, 2, 128, 128], MMDT, isOutput=False)   # C experts blockdiag per half
    wg = nc.declare_dram_parameter("wg", [KD, 128, E], MMDT, isOutput=False)       # G^T k-blocked
    wb = nc.declare_dram_parameter("wb", [128, L, KD], F32, isOutput=False)        # bias cols (+1 on l=0)
    we = nc.declare_dram_parameter("we", [4, ER + 4], MMDT, isOutput=False)        # expert bcast mask | ones
    wi = nc.declare_dram_parameter("wi", [128, 128], F16, isOutput=False)          # identity (PE transpose)

    def mm(out, lhsT, rhs, start, stop):
        nc.tensor.matmul(out, lhsT, rhs, start=start, stop=stop)

    with TileContext(nc) as tc:
        with (
            tc.tile_pool(name="wpool", bufs=1) as wpool,
            tc.tile_pool(name="xpool", bufs=2) as xpool,
            tc.tile_pool(name="apool", bufs=2) as apool,
            tc.tile_pool(name="mpool", bufs=2) as mpool,
            tc.tile_pool(name="spool", bufs=2) as spool,
            tc.tile_pool(name="pbig", bufs=2, space="PSUM") as pbig,
            tc.tile_pool(name="puv", bufs=2, space="PSUM") as puv,
            tc.tile_pool(name="ptp", bufs=2, space="PSUM") as ptp,
        ):
            # ---- weights to SBUF (once) ----
            vsb = wpool.tile([128, L, KD, ER], MMDT)
            usb = wpool.tile([128, L, 2, D], MMDT)
            csb = wpool.tile([128, L, 2, 128], MMDT)
            gsb = wpool.tile([128, KD, E], MMDT)
            bsb = wpool.tile([128, L, KD], F32)
            esb = wpool.tile([4, ER + 4], MMDT)
            isb = wpool.tile([128, 128], F16)
            negone = wpool.tile([128, 1], F32)
            nc.vector.memset(negone, -1.0)
            qeps = wpool.tile([128, 1], F32)
            nc.vector.memset(qeps, 1e-30)
            for l in range(L):
                nc.sync.dma_start(out=vsb[:, l], in_=wv[l].rearrange("k p m -> p k m"))
                nc.sync.dma_start(out=usb[:, l], in_=wu[l].rearrange("c p d -> p c d"))
                nc.sync.dma_start(out=csb[:, l], in_=wc[l].rearrange("h p m -> p h m"))
            nc.sync.dma_start(out=gsb, in_=wg.rearrange("k p e -> p k e"))
            nc.sync.dma_start(out=bsb, in_=wb[:])
            nc.sync.dma_start(out=esb, in_=we[:])
            nc.sync.dma_start(out=isb, in_=wi[:])

            for c in range(cb):
                # ---- load batch-major int8 rows, dequant, transpose ----
                xb = xpool.tile([128, jt, D], I8, tag="xb", name=f"xb_{c}")
                nc.sync.dma_start(out=xb, in_=x_in[c].rearrange("j p d -> p j d"))
                xscb = spool.tile([128, jt], F32, tag="xsc", name=f"xs_{c}")
                nc.sync.dma_start(out=xscb, in_=x_sc[c])
                xh = xpool.tile([128, jt, D], F16, tag="xh", name=f"xh_{c}")
                for j in range(jt):
                    nc.scalar.activation(xh[:, j], xb[:, j], AF.Identity,
                                         scale=xscb[:, j:j + 1])
                x0 = xpool.tile([128, KD, nb], MMDT, tag="x0", name=f"x0_{c}")
                for k in range(KD):
                    pt = ptp.tile([128, nb], F16, tag="tp", name=f"tin_{c}_{k}")
                    for j in range(jt):
                        nc.tensor.transpose(
                            pt[:, j * 128:(j + 1) * 128],
                            xh[:, j, k * 128:(k + 1) * 128], isb)
                    nc.scalar.activation(x0[:, k], pt, AF.Copy)
                a1 = apool.tile([128, KD, nb], F32, tag="a1", name=f"a1_{c}")
                xi = x0
                for l in range(L):
                    # ---- gating: scores = softmax over E of xi @ G^T ----
                    g_ps = puv.tile([128, nb], F32, tag="uv", name=f"g_{c}_{l}")
                    for k in range(KD):
                        mm(g_ps[0:4], gsb[:, k], xi[:, k], k == 0, k == KD - 1)
                    p_sb = spool.tile([4, nb], MMDT, tag="p", name=f"p_{c}_{l}")
                    nc.scalar.activation(p_sb, g_ps[0:4], AF.Exp)
                    z_ps = puv.tile([128, nb], F32, tag="uv", name=f"z_{c}_{l}")
                    mm(z_ps[0:1], esb[:, ER:ER + 1], p_sb, True, True)
                    rinv = spool.tile([1, nb], MMDT, tag="rinv", name=f"r_{c}_{l}")
                    with nc.allow_low_precision(reason="fp16 softmax denom"):
                        nc.vector.reciprocal(out=rinv, in_=z_ps[0:1])
                    rb_ps = puv.tile([128, nb], F32, tag="uv", name=f"rb_{c}_{l}")
                    mm(rb_ps[0:4], esb[0:1, ER:ER + 4], rinv, True, True)
                    s_sb = spool.tile([4, nb], MMDT, tag="s", name=f"s_{c}_{l}")
                    nc.vector.tensor_mul(s_sb, p_sb, rb_ps[0:4])
                    # broadcast scores over each expert's R rows: [4,nb]->[256,nb]
                    sb_ps = pbig.tile([128, 2, nb], F32, tag="big", name=f"sb_{c}_{l}")
                    for h in range(2):
                        mm(sb_ps[:, h], esb[:, h * 128:(h + 1) * 128], s_sb, True, True)
                    sbig = mpool.tile([128, 2, nb], MMDT, tag="sbig", name=f"sg_{c}_{l}")
                    nc.vector.tensor_copy(sbig, sb_ps)
                    # ---- v = tanh(xi @ Vcat) ----
                    v_ps = pbig.tile([128, 2, nb], F32, tag="big", name=f"v_{c}_{l}")
                    for h in range(2):
                        for k in range(KD):
                            mm(v_ps[:, h], vsb[:, l, k, h * 128:(h + 1) * 128],
                               xi[:, k], k == 0, k == KD - 1)
                    vt = mpool.tile([128, 2, nb], MMDT, tag="vt", name=f"vt_{c}_{l}")
                    nc.scalar.activation(vt, v_ps, AF.Tanh)
                    # ---- w = tanh(v @ C) per expert (2x2 packed) ----
                    w_ps = pbig.tile([128, 2, nb], F32, tag="big", name=f"w_{c}_{l}")
                    for h in range(2):
                        mm(w_ps[:, h], csb[:, l, h], vt[:, h], True, True)
                    wt = mpool.tile([128, 2, nb], MMDT, tag="wt", name=f"wt_{c}_{l}")
                    nc.scalar.activation(wt, w_ps, AF.Tanh)
                    # ---- fold scores: wp = wt * sbig  (gpsimd, all-SBUF) ----
                    wp = mpool.tile([128, 2, nb], MMDT, tag="wp", name=f"wp_{c}_{l}")
                    nc.gpsimd.tensor_mul(wp, wt, sbig)
                    # ---- uvmix = wp @ Ucat^T ; A1 accumulation ----
                    for m in range(KD):
                        uv_ps = puv.tile([128, nb], F32, tag="uv", name=f"uv_{c}_{l}_{m}")
                        for h in range(2):
                            mm(uv_ps, usb[:, l, h, m * 128:(m + 1) * 128],
                               wp[:, h], h == 0, h == 1)
                        if l == 0:
                            # A1 = uv + (1 + b_0)
                            nc.scalar.activation(a1[:, m], uv_ps, AF.Identity,
                                                 bias=bsb[:, 0, m:m + 1])
                        else:
                            # A1 = (uv + b_l) + A1
                            nc.vector.scalar_tensor_tensor(
                                out=a1[:, m], in0=uv_ps, scalar=bsb[:, l, m:m + 1],
                                in1=a1[:, m], op0=ALU.add, op1=ALU.add)
                    # ---- xi = x0 * A1 (gpsimd, chunk-wise to pipeline) ----
                    if l < L - 1:
                        xo = xpool.tile([128, KD, nb], MMDT, tag="xi", name=f"xi_{c}_{l}")
                        for m in range(KD):
                            nc.gpsimd.tensor_mul(xo[:, m], x0[:, m], a1[:, m])
                        xi = xo
                # ---- s = A1 - 1; transpose to batch-major ----
                sf = xpool.tile([128, KD, nb], F16, tag="xi", name=f"sf_{c}")
                for m in range(KD):
                    nc.scalar.activation(sf[:, m], a1[:, m], AF.Identity,
                                         bias=negone[:, 0:1])
                yb = mpool.tile([128, jt, 2, nb], F16, tag="yb", name=f"yb_{c}")
                for j in range(jt):
                    for h in range(2):
                        pt = ptp.tile([128, nb], F16, tag="tp", name=f"tout_{c}_{j}_{h}")
                        for kk in range(jt):
                            nc.tensor.transpose(
                                pt[:, kk * 128:(kk + 1) * 128],
                                sf[:, h * jt + kk, j * 128:(j + 1) * 128], isb)
                        nc.scalar.activation(yb[:, j, h], pt, AF.Copy)
                # ---- per-row int8 quantization: q = round(s / scl), scl = rowmax/127
                rmax = spool.tile([128, jt], F32, tag="rmax", name=f"rm_{c}")
                nc.vector.tensor_reduce(rmax, yb, mybir.AxisListType.XY,
                                        ALU.max, apply_absolute_value=True)
                scl = spool.tile([128, jt], F32, tag="scl", name=f"sc_{c}")
                nc.scalar.activation(scl, rmax, AF.Identity,
                                     bias=qeps[:, 0:1], scale=1.0 / 127.0)
                rinv = spool.tile([128, jt], F32, tag="rin", name=f"ri_{c}")
                with nc.allow_low_precision(reason="int8 quant scale"):
                    nc.vector.reciprocal(out=rinv, in_=scl)
                q8 = mpool.tile([128, jt, 2, nb], I8, tag="q8", name=f"q8_{c}")
                for j in range(jt):
                    nc.scalar.activation(q8[:, j], yb[:, j], AF.Identity,
                                         scale=rinv[:, j:j + 1])
                nc.sync.dma_start(out=y_out[c].rearrange("j p h n -> p j h n"), in_=q8)
                nc.sync.dma_start(out=y_sc[c], in_=scl)
    nc.compile()
    return nc


# ---------------- host side ----------------


def prep_weights(U, V, C, biases, G):
    U = np.asarray(U, np.float32)
    V = np.asarray(V, np.float32)
    C = np.asarray(C, np.float32)
    biases = np.asarray(biases, np.float32)
    G = np.asarray(G, np.float32)
    wv = np.ascontiguousarray(
        V.transpose(0, 2, 1, 3).reshape(L, D, ER).reshape(L, KD, 128, ER)
    ).astype(np.float16)
    wu = np.ascontiguousarray(
        U.transpose(0, 1, 3, 2).reshape(L, ER, D).reshape(L, 2, 128, D)
    ).astype(np.float16)
    wc = np.zeros((L, 2, 128, 128), np.float16)
    for l in range(L):
        for h in range(2):
            wc[l, h, 0:64, 0:64] = C[l, 2 * h]
            wc[l, h, 64:128, 64:128] = C[l, 2 * h + 1]
    wg = np.ascontiguousarray(G.T.reshape(KD, 128, E)).astype(np.float16)
    ball = biases.copy()
    ball[0] += 1.0
    wb = np.ascontiguousarray(ball.reshape(L, KD, 128).transpose(2, 0, 1))
    we = np.zeros((4, ER + 4), np.float16)
    for e in range(E):
        we[e, e * R:(e + 1) * R] = 1.0
    we[:, ER:] = 1.0
    wid = np.eye(128, dtype=np.float16)
    return dict(wv=wv, wu=wu, wc=wc, wg=wg, wb=wb, we=we, wi=wid)


class _Runner:
    """Persistent-jit SPMD dispatcher for the bass program.

    Mirrors concourse.bass2jax.run_bass_via_pjrt's lowering (same
    _bass_exec_p custom call inside a shard_map) but keeps the jitted
    executable, the weights, and the output zero-buffers device-resident so
    that per call only x (up) and y (down) cross the axon tunnel.
    """

    def __init__(self, nc):
        import jax
        from jax.sharding import Mesh, PartitionSpec, NamedSharding
        from jax.experimental.shard_map import shard_map
        from concourse import bass2jax

        self.jax = jax
        self.nc = nc
        bass2jax.install_neuronx_cc_hook()

        partition_name = (
            nc.partition_id_tensor.name if nc.partition_id_tensor else None
        )
        in_names, out_names, out_avals, zero_outs = [], [], [], []
        for alloc in nc.m.functions[0].allocations:
            if not isinstance(alloc, mybir.MemoryLocationSet):
                continue
            name = alloc.memorylocations[0].name
            if alloc.kind == "ExternalInput":
                if name != partition_name:
                    in_names.append(name)
            elif alloc.kind == "ExternalOutput":
                out_names.append(name)
                shape = tuple(alloc.tensor_shape)
                dtype = mybir.dt.np(alloc.dtype)
                out_avals.append(jax.core.ShapedArray(shape, dtype))
                zero_outs.append(np.zeros(shape, dtype))
        n_params = len(in_names)
        all_names = in_names + out_names
        if partition_name is not None:
            all_names = all_names + [partition_name]
        all_names = tuple(all_names)
        out_avals = tuple(out_avals)

        def _body(*args):
            operands = list(args)
            if partition_name is not None:
                operands.append(bass2jax.partition_id_tensor())
            outs = bass2jax._bass_exec_p.bind(
                *operands,
                out_avals=out_avals,
                in_names=all_names,
                out_names=tuple(out_names),
                lowering_input_output_aliases=(),
                sim_require_finite=True,
                sim_require_nnan=True,
                nc=nc,
            )
            return tuple(outs)

        devices = jax.devices()[:NCORES]
        assert len(devices) == NCORES
        self.mesh = Mesh(np.asarray(devices), ("core",))
        self.sh = NamedSharding(self.mesh, PartitionSpec("core"))
        n_args = n_params + len(out_names)
        self.fn = jax.jit(
            shard_map(
                _body, mesh=self.mesh,
                in_specs=(PartitionSpec("core"),) * n_args,
                out_specs=(PartitionSpec("core"),) * len(out_names),
                check_rep=False,
            ),
            keep_unused=True,
        )
        self.in_names = in_names
        self.zero_outs = zero_outs
        self.w_dev = None
        self.z_dev = None
        self._w_host = None

    def weights_match(self, w: dict):
        return self._w_host is not None and all(
            np.array_equal(w[n], self._w_host[n]) for n in w)

    def set_weights(self, w: dict):
        """Upload weights + output buffers once; device-resident thereafter."""
        self._w_host = {n: a.copy() for n, a in w.items()}
        jax = self.jax
        self.w_dev = {
            n: jax.device_put(
                np.broadcast_to(w[n], (NCORES,) + w[n].shape).reshape(
                    (NCORES * w[n].shape[0],) + w[n].shape[1:]), self.sh)
            for n in self.in_names if n not in ("x_in", "x_sc")
        }
        self.z_dev = [
            jax.device_put(
                np.zeros((NCORES * z.shape[0],) + z.shape[1:], z.dtype), self.sh)
            for z in self.zero_outs
        ]
        for a in list(self.w_dev.values()) + self.z_dev:
            a.block_until_ready()

    def launch(self, q8_piece, xsc_piece):
        """Dispatch one piece; returns the (async) sharded output arrays."""
        xd = self.jax.device_put(
            q8_piece.reshape(NCORES * CBP, JT, 128, D), self.sh)
        xsd = self.jax.device_put(
            xsc_piece.reshape(NCORES * CBP, JT, 128).transpose(0, 2, 1), self.sh)
        per = {"x_in": xd, "x_sc": xsd}
        args = [per.get(n) if n in per else self.w_dev[n] for n in self.in_names]
        return self.fn(*args, *self.z_dev)


_NC = None
_RUNNER = None


def _get_runner():
    global _NC, _RUNNER
    if _NC is None:
        _NC = build_nc(PBS, NB)
    if _RUNNER is None:
        _RUNNER = _Runner(_NC)
    return _RUNNER


def _quantize_rows(xs):
    """Per-row int8 quantization of x: returns (q8 [rows, D], scales [rows])."""
    m = np.abs(xs).max(axis=1)
    scl = m / 127.0 + 1e-30
    q = np.multiply(xs, (1.0 / scl)[:, None], dtype=np.float32)
    np.rint(q, out=q)
    return q.astype(np.int8), scl.astype(np.float32)


def _reconstruct(y_slice, x_slice, q, scl):
    """y = x * (1 + q * rowscale): q int8 [rows, D], scl [n, 128, JT] f32."""
    rows = x_slice.shape[0]
    scale_rows = scl.transpose(0, 2, 1).reshape(rows)
    t = np.multiply(q.reshape(rows, D), scale_rows[:, None],
                    dtype=np.float32, casting="unsafe")  # fused int8->f32 * scale
    t += 1.0
    np.multiply(x_slice, t, out=y_slice)


def _kernel_fallback(x, w):
    """Plain run_bass_kernel_spmd path (re-ships everything; slower)."""
    global _NC
    if _NC is None:
        _NC = build_nc(PBS, NB)
    y = np.empty((B, D), np.float32)
    for p in range(PIECES):
        q8, xscl = _quantize_rows(x[p * PR:(p + 1) * PR])
        xp = q8.reshape(NCORES, CBP, JT, 128, D)
        xsp = xscl.reshape(NCORES, CBP, JT, 128).transpose(0, 1, 3, 2)
        in_maps = [dict(w, x_in=xp[c], x_sc=np.ascontiguousarray(xsp[c]))
                   for c in range(NCORES)]
        res = run_bass_kernel_spmd(_NC, in_maps, core_ids=list(range(NCORES)),
                                   trace=False)
        for c in range(NCORES):
            lo = p * PR + c * PBS
            _reconstruct(y[lo:lo + PBS], x[lo:lo + PBS],
                         res.results[c]["y_out"], res.results[c]["y_sc"])
    return y


def kernel(x, U, V, C, biases, G, _trace=False, _nb=NB):
    x = np.asarray(x, np.float32)
    w = prep_weights(U, V, C, biases, G)
    try:
        runner = _get_runner()
        t0 = time.time()
        if runner.w_dev is None or not runner.weights_match(w):
            runner.set_weights(w)
        # producer thread uploads+dispatches pieces; consumer (main) fetches
        # + reconstructs — up/down overlap on the duplex tunnel.
        import queue as _queue
        import threading
        timing = []
        qch = _queue.Queue()

        def _producer():
            prev = None
            for p in range(PIECES):
                q8, xscl = _quantize_rows(x[p * PR:(p + 1) * PR])
                t1 = time.time()
                out = runner.launch(q8, xscl)
                if prev is not None:
                    # wait for previous piece's exec before queuing the next
                    # upload, so its download interleaves with this upload
                    prev[1].block_until_ready()
                for o in out:
                    try:
                        o.copy_to_host_async()
                    except Exception:
                        pass
                timing.append(("launch%d" % p, round(time.time() - t1, 3)))
                qch.put((p, out))
                prev = out

        th = threading.Thread(target=_producer, daemon=True)
        th.start()
        y = np.empty((B, D), np.float32)
        for _ in range(PIECES):
            p, out = qch.get()
            t1 = time.time()
            q = np.asarray(out[0])    # [NCORES*CBP, JT, 128, 2, NB] int8
            scl = np.asarray(out[1])  # [NCORES*CBP, 128, JT] f32
            _reconstruct(y[p * PR:(p + 1) * PR], x[p * PR:(p + 1) * PR], q, scl)
            timing.append(("fetch%d" % p, round(time.time() - t1, 3)))
        th.join()
        kernel.last_run_wall_s = time.time() - t0
        kernel.last_timing = timing
    except Exception:
        import traceback
        traceback.print_exc()
        t0 = time.time()
        y = _kernel_fallback(x, w)
        kernel.last_run_wall_s = time.time() - t0
    return y
